# revision 1
# baseline (speedup 1.0000x reference)
"""KoLeo loss kernel for Trainium2, 8 NeuronCores.

Strategy (data-parallel brute-force 1-NN over L2-normalized rows):
  - Each core gets a row-PERMUTED copy of x with its own 1024 rows first, so
    the self-match diagonal always falls in columns 0..1023 (core-invariant
    program, as required by SPMD).
  - On device: normalize rows in f32, cast to fp16, DMA-roundtrip through DRAM
    with XBAR transpose to build xT [128, 6, 8192] (D on partitions).
  - dots slab = xT_own_cols^T @ xT (fp16 matmul, f32 PSUM accumulate).
    Diagonal masked by adding -2*I at the known chunk/offset; running
    elementwise max over 512-col chunks; final row-max m.
  - pdist for normalized vectors: dist = sqrt(2 - 2*m)  (the reference's +EPS
    inside the diff perturbs the scalar loss by ~1e-8 relative - negligible).
  - loss partial per core = sum(log(dist + EPS)); host combines:
    loss = -(sum partials) / 8192.
"""

import sys

sys.path.insert(0, "/opt/trn_rl_repo")

import numpy as np

import concourse.bass as bass
import concourse.mybir as mybir
import concourse.tile as tile
from concourse import bacc
from concourse.bass_utils import run_bass_kernel_spmd

B = 8192
D = 768
NCORES = 8
RPC = B // NCORES  # 1024 rows per core
P = 128
KC = D // P  # 6 contraction chunks
CH = 512  # moving chunk width
NCH = B // CH  # 16 chunks
T = RPC // P  # 8 row tiles per core
EPS = 1e-8

f32 = mybir.dt.float32
f16 = mybir.dt.float16
AF = mybir.ActivationFunctionType
ALU = mybir.AluOpType
AX = mybir.AxisListType


def _build_program():
    nc = bacc.Bacc("TRN2", target_bir_lowering=False, debug=False, enable_asserts=True)
    x_in = nc.dram_tensor("xm", [B, D], f32, kind="ExternalInput").ap()
    mask4_in = nc.dram_tensor("mask4", [P, 4, CH], f32, kind="ExternalInput").ap()
    ones_in = nc.dram_tensor("ones", [P, 1], f32, kind="ExternalInput").ap()
    consts_in = nc.dram_tensor("consts", [P, 2], f32, kind="ExternalInput").ap()
    out_t = nc.dram_tensor("partial", [1, 1], f32, kind="ExternalOutput").ap()

    with tile.TileContext(nc) as tc:
        with (
            tc.tile_pool(name="big", bufs=1) as big,
            tc.tile_pool(name="work", bufs=3) as work,
            tc.tile_pool(name="psum", bufs=4, space="PSUM") as psum_pool,
            tc.tile_pool(name="dram", bufs=1, space="DRAM") as dram_pool,
        ):
            # persistent tiles
            xTn = [big.tile([P, KC, CH], f16, name=f"xT{n}", tag=f"xT{n}") for n in range(NCH)]
            mask4 = big.tile([P, 4, CH], f32, tag="mask4")
            ones = big.tile([P, 1], f32, tag="ones")
            consts = big.tile([P, 2], f32, tag="consts")
            accs = [big.tile([P, CH], f32, name=f"acc{t}", tag=f"acc{t}") for t in range(T)]

            nc.sync.dma_start(mask4[:], mask4_in)
            nc.sync.dma_start(ones[:], ones_in)
            nc.sync.dma_start(consts[:], consts_in)
            two = consts[:, 0:1]
            epsb = consts[:, 1:2]

            xn_dram = dram_pool.tile([B, D], f16, name="xn_dram")

            # Phase A: normalize rows -> fp16 -> DRAM; per 512-row chunk,
            # 6 transposing DMAs back into xTn[chunk].
            for j in range(B // P):  # 64 row tiles
                rt = work.tile([P, D], f32, tag="rt")
                nc.sync.dma_start(rt[:], x_in[j * P : (j + 1) * P, :])
                sq = work.tile([P, D], f32, tag="sq")
                ss = work.tile([P, 1], f32, tag="ss")
                nc.scalar.activation(sq[:], rt[:], AF.Square, accum_out=ss[:])
                nrm = work.tile([P, 1], f32, tag="nrm")
                nc.scalar.activation(nrm[:], ss[:], AF.Sqrt)
                nrmc = work.tile([P, 1], f32, tag="nrmc")
                nc.vector.tensor_scalar_max(nrmc[:], nrm[:], float(EPS))
                rinv = work.tile([P, 1], f32, tag="rinv")
                nc.vector.reciprocal(rinv[:], nrmc[:])
                xn = work.tile([P, D], f16, tag="xn")
                nc.scalar.mul(xn[:], rt[:], rinv[:])
                nc.sync.dma_start(xn_dram[j * P : (j + 1) * P, :], xn[:])
                if j % 4 == 3:
                    n = j // 4
                    for c in range(KC):
                        nc.sync.dma_start_transpose(
                            xTn[n][:, c, :],
                            xn_dram[n * CH : (n + 1) * CH, c * P : (c + 1) * P],
                        )

            # Phase B: matmul + running max
            for n in range(NCH):
                for t in range(T):
                    pt = psum_pool.tile([P, CH], f32, tag="pt")
                    for c in range(KC):
                        nt = t // 4  # chunk holding this row tile's columns
                        nc.tensor.matmul(
                            pt[:],
                            lhsT=xTn[nt][:, c, (t % 4) * P : (t % 4 + 1) * P],
                            rhs=xTn[n][:, c, :],
                            start=(c == 0),
                            stop=(c == KC - 1),
                        )
                    if n == t // 4:
                        v = t % 4
                        if n == 0:
                            nc.vector.tensor_tensor(
                                out=accs[t][:], in0=pt[:], in1=mask4[:, v],
                                op=ALU.add,
                            )
                        else:
                            tmp = work.tile([P, CH], f32, tag="tmp")
                            nc.vector.tensor_tensor(
                                out=tmp[:], in0=pt[:], in1=mask4[:, v], op=ALU.add
                            )
                            nc.vector.tensor_tensor(
                                out=accs[t][:], in0=accs[t][:], in1=tmp[:],
                                op=ALU.max,
                            )
                    elif n == 0:
                        nc.vector.tensor_copy(out=accs[t][:], in_=pt[:])
                    else:
                        nc.vector.tensor_tensor(
                            out=accs[t][:], in0=accs[t][:], in1=pt[:], op=ALU.max
                        )

            # Phase C: row max -> dist -> log -> partial sum
            rmax = big.tile([P, T], f32, tag="rmax")
            for t in range(T):
                nc.vector.tensor_reduce(
                    rmax[:, t : t + 1], accs[t][:], axis=AX.X, op=ALU.max
                )
            dist = big.tile([P, T], f32, tag="dist")
            nc.scalar.activation(dist[:], rmax[:], AF.Sqrt, scale=-2.0, bias=two)
            logd = big.tile([P, T], f32, tag="logd")
            lsum = big.tile([P, 1], f32, tag="lsum")
            nc.scalar.activation(
                logd[:], dist[:], AF.Ln, bias=epsb, accum_out=lsum[:]
            )
            pfin = psum_pool.tile([1, 1], f32, tag="pfin")
            nc.tensor.matmul(pfin[:], lhsT=ones[:], rhs=lsum[:], start=True, stop=True)
            res = big.tile([1, 1], f32, tag="res")
            nc.vector.tensor_copy(out=res[:], in_=pfin[:])
            nc.sync.dma_start(out_t[:], res[:])

    nc.compile()
    return nc


_NC_CACHE = None


def _get_nc():
    global _NC_CACHE
    if _NC_CACHE is None:
        _NC_CACHE = _build_program()
    return _NC_CACHE


def _make_in_maps(x: np.ndarray):
    mask4 = np.zeros((P, 4, CH), dtype=np.float32)
    for v in range(4):
        mask4[:, v, v * P : (v + 1) * P] = -2.0 * np.eye(P, dtype=np.float32)
    ones = np.ones((P, 1), dtype=np.float32)
    consts = np.zeros((P, 2), dtype=np.float32)
    consts[:, 0] = 2.0
    consts[:, 1] = EPS
    in_maps = []
    for m in range(NCORES):
        own = x[m * RPC : (m + 1) * RPC]
        rest = np.concatenate([x[: m * RPC], x[(m + 1) * RPC :]], axis=0)
        xm = np.ascontiguousarray(np.concatenate([own, rest], axis=0))
        in_maps.append({"xm": xm, "mask4": mask4, "ones": ones, "consts": consts})
    return in_maps


def kernel(student_output: np.ndarray) -> np.ndarray:
    x = np.asarray(student_output, dtype=np.float32)
    nc = _get_nc()
    in_maps = _make_in_maps(x)
    res = run_bass_kernel_spmd(nc, in_maps, list(range(NCORES)))
    total = 0.0
    for r in res.results:
        total += float(r["partial"].reshape(()))
    loss = -(total / B)
    return np.float32(loss)



# revision 8
# speedup vs baseline: 2.1584x; 2.1584x over previous
"""KoLeo loss kernel for Trainium2, 8 NeuronCores.

Strategy (data-parallel brute-force 1-NN over L2-normalized rows):
  - Each core gets a row-PERMUTED copy of x with its own 1024 rows first, so
    the self-match diagonal always falls in columns 0..1023 (core-invariant
    program, as required by SPMD).
  - On device: normalize rows in f32, cast to f16 (or fp8e4 scaled by S),
    transpose on-chip via identity matmuls on the PE (f32 PSUM -> cast drain),
    building xT [128, 6, 8192] with D on partitions. No DRAM roundtrip.
  - dots slab = xT_own^T @ xT accumulated over 6 K-chunks in PSUM. The
    self-match diagonal is masked by a 7th matmul in the accumulation group
    (lhsT = -2*S^2*I, rhs = one-hot block) so no extra vector work.
  - Running elementwise max over 512-col chunks on DVE; final row-max m.
  - pdist for normalized vectors: dist = sqrt(2 - 2*m/S^2)  (the reference's
    +EPS inside the diff perturbs the scalar loss by ~1e-8 rel - negligible).
  - loss partial per core = sum(log(dist + EPS)); host combines:
    loss = -(sum partials) / 8192.
"""

import sys

sys.path.insert(0, "/opt/trn_rl_repo")

import numpy as np

import concourse.bass as bass
import concourse.mybir as mybir
import concourse.tile as tile
from concourse import bacc
from concourse.bass_utils import run_bass_kernel_spmd

B = 8192
D = 768
NCORES = 8
RPC = B // NCORES  # 1024 rows per core
P = 128
KC = D // P  # 6 contraction chunks
CH = 512  # moving chunk width
NCH = B // CH  # 16 chunks
T = RPC // P  # 8 row tiles per core
NJ = B // P  # 64 row tiles of the full x
EPS = 1e-8

USE_FP8 = False
S = 8.0 if USE_FP8 else 1.0  # prescale for fp8 dynamic range

f32 = mybir.dt.float32
f16 = mybir.dt.float16
f8 = mybir.dt.float8e4
MMDT = f8 if USE_FP8 else f16
AF = mybir.ActivationFunctionType
ALU = mybir.AluOpType
AX = mybir.AxisListType
PM = mybir.MatmulPerfMode


def _build_program():
    nc = bacc.Bacc("TRN2", target_bir_lowering=False, debug=False, enable_asserts=True)
    x_in = nc.dram_tensor("xm", [B, D], f32, kind="ExternalInput").ap()
    ident_in = nc.dram_tensor("ident", [P, P], MMDT, kind="ExternalInput").ap()
    mdiag_in = nc.dram_tensor("mdiag", [P, P], MMDT, kind="ExternalInput").ap()
    monehot_in = nc.dram_tensor("monehot", [P, 4, CH], MMDT, kind="ExternalInput").ap()
    ones_in = nc.dram_tensor("ones", [P, 1], f32, kind="ExternalInput").ap()
    consts_in = nc.dram_tensor("consts", [P, 2], f32, kind="ExternalInput").ap()
    out_t = nc.dram_tensor("partial", [1, 1], f32, kind="ExternalOutput").ap()

    with tile.TileContext(nc) as tc:
        with (
            tc.tile_pool(name="big", bufs=1) as big,
            tc.tile_pool(name="work", bufs=3) as work,
            tc.tile_pool(name="small", bufs=4) as small,
            tc.tile_pool(name="pmm", bufs=4, space="PSUM") as pmm,
            tc.tile_pool(name="ptr", bufs=3, space="PSUM") as ptr,
            tc.tile_pool(name="pfi", bufs=1, space="PSUM") as pfi,
        ):
            # persistent tiles
            xT = big.tile([P, KC, B], MMDT, tag="xT")
            ident = big.tile([P, P], MMDT, tag="ident")
            mdiag = big.tile([P, P], MMDT, tag="mdiag")
            monehot = big.tile([P, 4, CH], MMDT, tag="monehot")
            ones = big.tile([P, 1], f32, tag="ones")
            consts = big.tile([P, 2], f32, tag="consts")
            accs = [big.tile([P, CH], f32, name=f"acc{t}", tag=f"acc{t}") for t in range(T)]

            nc.sync.dma_start(ident[:], ident_in)
            nc.sync.dma_start(mdiag[:], mdiag_in)
            nc.sync.dma_start(monehot[:], monehot_in)
            nc.sync.dma_start(ones[:], ones_in)
            nc.sync.dma_start(consts[:], consts_in)
            two = consts[:, 0:1]
            epsb = consts[:, 1:2]

            def emit_chunk(n):
                # dots block rows: all 8 own tiles, cols: chunk n (512 wide)
                for t in range(T):
                    pt = pmm.tile([P, CH], f32, tag="pt")
                    diag = n == t // 4
                    for c in range(KC):
                        nc.tensor.matmul(
                            pt[:],
                            lhsT=xT[:, c, t * P : (t + 1) * P],
                            rhs=xT[:, c, n * CH : (n + 1) * CH],
                            start=(c == 0),
                            stop=(c == KC - 1 and not diag),
                        )
                    if diag:
                        # mask self-match: adds -2*S^2 at [p, (t%4)*128+p]
                        nc.tensor.matmul(
                            pt[:],
                            lhsT=mdiag[:],
                            rhs=monehot[:, t % 4, :],
                            start=False,
                            stop=True,
                        )
                    if n == 0:
                        nc.vector.tensor_copy(out=accs[t][:], in_=pt[:])
                    else:
                        nc.vector.tensor_tensor(
                            out=accs[t][:], in0=accs[t][:], in1=pt[:], op=ALU.max
                        )

            # Phase A pipeline: load -> normalize -> PE transpose; matmul chunks
            # are interleaved as soon as their xT columns are complete.
            for j in range(NJ):
                rt = work.tile([P, D], f32, tag="rt")
                nc.sync.dma_start(rt[:], x_in[j * P : (j + 1) * P, :])
                sq = work.tile([P, D], f32, tag="sq")
                ss = small.tile([P, 1], f32, tag="ss")
                nc.scalar.activation(sq[:], rt[:], AF.Square, accum_out=ss[:])
                nrm = small.tile([P, 1], f32, tag="nrm")
                # norm/S (prescale folded into the sqrt)
                nc.scalar.activation(nrm[:], ss[:], AF.Sqrt, scale=1.0 / (S * S))
                nrmc = small.tile([P, 1], f32, tag="nrmc")
                nc.vector.tensor_scalar_max(nrmc[:], nrm[:], float(EPS))
                rinv = small.tile([P, 1], f32, tag="rinv")
                nc.vector.reciprocal(rinv[:], nrmc[:])
                xn = work.tile([P, D], MMDT, tag="xn")
                nc.scalar.mul(xn[:], rt[:], rinv[:])
                # transpose 6 chunks via identity matmuls; pack 4+2 per PSUM
                # bank so each drain is one wide copy
                pta = ptr.tile([P, 4, P], f32, tag="pta")
                ptb_full = ptr.tile([P, 4, P], f32, tag="pta")
                ptb = ptb_full[:, 0:2, :]
                for c in range(KC):
                    dst = pta[:, c, :] if c < 4 else ptb_full[:, c - 4, :]
                    nc.tensor.matmul(
                        dst,
                        lhsT=xn[:, c * P : (c + 1) * P],
                        rhs=ident[:],
                        start=True,
                        stop=True,
                    )
                # split transpose drains between scalar and vector
                eng_a = nc.scalar if j % 2 == 0 else nc.vector
                eng_b = nc.vector if j % 2 == 0 else nc.scalar
                if eng_a is nc.scalar:
                    nc.scalar.copy(out=xT[:, 0:4, j * P : (j + 1) * P], in_=pta[:])
                else:
                    nc.vector.tensor_copy(
                        out=xT[:, 0:4, j * P : (j + 1) * P], in_=pta[:]
                    )
                if eng_b is nc.scalar:
                    nc.scalar.copy(out=xT[:, 4:6, j * P : (j + 1) * P], in_=ptb[:])
                else:
                    nc.vector.tensor_copy(
                        out=xT[:, 4:6, j * P : (j + 1) * P], in_=ptb[:]
                    )
                if j == T - 1:
                    emit_chunk(0)
                    emit_chunk(1)
                elif j > T - 1 and j % 4 == 3:
                    emit_chunk(j // 4)

            # Phase C: row max -> dist -> log -> partial sum
            rmax = big.tile([P, T], f32, tag="rmax")
            for t in range(T):
                nc.vector.tensor_reduce(
                    rmax[:, t : t + 1], accs[t][:], axis=AX.X, op=ALU.max
                )
            dist = big.tile([P, T], f32, tag="dist")
            nc.scalar.activation(
                dist[:], rmax[:], AF.Sqrt, scale=-2.0 / (S * S), bias=two
            )
            logd = big.tile([P, T], f32, tag="logd")
            lsum = big.tile([P, 1], f32, tag="lsum")
            nc.scalar.activation(
                logd[:], dist[:], AF.Ln, bias=epsb, accum_out=lsum[:]
            )
            pfin = pfi.tile([1, 1], f32, tag="pfin")
            nc.tensor.matmul(pfin[:], lhsT=ones[:], rhs=lsum[:], start=True, stop=True)
            res = big.tile([1, 1], f32, tag="res")
            nc.vector.tensor_copy(out=res[:], in_=pfin[:])
            nc.sync.dma_start(out_t[:], res[:])

    nc.compile()
    return nc


_NC_CACHE = None


def _get_nc():
    global _NC_CACHE
    if _NC_CACHE is None:
        _NC_CACHE = _build_program()
    return _NC_CACHE


def _np_dtype():
    if USE_FP8:
        import ml_dtypes

        return ml_dtypes.float8_e4m3
    return np.float16


def _make_in_maps(x: np.ndarray):
    dt = _np_dtype()
    ident = np.eye(P, dtype=np.float32).astype(dt)
    mdiag = (-2.0 * S * S * np.eye(P, dtype=np.float32)).astype(dt)
    monehot = np.zeros((P, 4, CH), dtype=np.float32)
    for v in range(4):
        monehot[:, v, v * P : (v + 1) * P] = np.eye(P, dtype=np.float32)
    monehot = monehot.astype(dt)
    ones = np.ones((P, 1), dtype=np.float32)
    consts = np.zeros((P, 2), dtype=np.float32)
    consts[:, 0] = 2.0
    consts[:, 1] = EPS
    in_maps = []
    for m in range(NCORES):
        own = x[m * RPC : (m + 1) * RPC]
        rest = np.concatenate([x[: m * RPC], x[(m + 1) * RPC :]], axis=0)
        xm = np.ascontiguousarray(np.concatenate([own, rest], axis=0))
        in_maps.append(
            {
                "xm": xm,
                "ident": ident,
                "mdiag": mdiag,
                "monehot": monehot,
                "ones": ones,
                "consts": consts,
            }
        )
    return in_maps


def kernel(student_output: np.ndarray) -> np.ndarray:
    x = np.asarray(student_output, dtype=np.float32)
    nc = _get_nc()
    in_maps = _make_in_maps(x)
    res = run_bass_kernel_spmd(nc, in_maps, list(range(NCORES)))
    total = 0.0
    for r in res.results:
        total += float(r["partial"].reshape(()))
    loss = -(total / B)
    return np.float32(loss)
